# revision 3
# baseline (speedup 1.0000x reference)
"""Trainium2 Bass kernel for CanonicalAlignmentLoss.

Strategy ("subject-grouped sharding", fp8, aligned 64-col tiles):
  - Host groups the N=524288 rows by subject id (16 subjects) and deals each
    subject's rows across the 8 cores, padding each (core, subject) segment
    with zero rows up to whole 128-row tiles so every SBUF tile is
    single-subject.
  - Data is quantized to fp8 e4m3 on host (mybir float8e4 == ml_dtypes
    float8_e4m3): quantization noise averages out over ~32k rows/subject and
    the diagonal bias cancels in the pairwise covariance differences
    (measured loss rel-err ~6e-4 vs the fp32 reference, tolerance 2e-2).
  - Row width is exactly 64 bytes (no ones column, no pad): with a 16B-aligned
    64-byte-stride moving operand the PE streams fp8 at ~2 cols/cycle, so each
    128-row tile costs ~15ns instead of ~30ns.  Per-subject row sums (needed
    for the mean term) are computed on host from the same quantized values.
  - Device (per core): stream the shard through SBUF in ~1MB chunks (DMA
    efficiency needs >=1MB transfers); for each 128-row tile run one matmul
    accumulating X^T X into that subject's PSUM block.
  - Host: sum the 8 per-core [64, 16, 64] partials, form covariances with the
    host-side sums/counts, and do the tiny [16,16] pairwise-Frobenius stage.
"""

import numpy as np
import ml_dtypes

import concourse.bass as bass
import concourse.tile as tile
from concourse import bacc, mybir
from concourse.bass_utils import run_bass_kernel_spmd

NCORES = 8
S = 16
D = 64
ROWW = 64  # bytes per row: 64 fp8 data cols, 16B-aligned
NCHUNKS = 4
OUTW = 64


def _build_nc(tiles_per_subject, nchunks, reps=1, bufs=4, ctile=None):
    """Build the SPMD Bass program (identical on all cores).

    reps>1 repeats the whole compute schedule for steady-state timing: each
    rep re-opens the PSUM accumulation groups (start=True) and the per-rep
    results are accumulated into SBUF with DVE adds (keeping every rep's
    matmuls live), so the host must divide the output by `reps`.
    """
    nc = bacc.Bacc("TRN2", target_bir_lowering=False, debug=False)
    T = sum(tiles_per_subject)
    if ctile is None:
        ctile = T // nchunks
    assert ctile * nchunks == T

    x = nc.declare_dram_parameter(
        "x", [nchunks, 128, ctile * ROWW], mybir.dt.float8e4, isOutput=False
    )
    out = nc.declare_dram_parameter(
        "out", [64, S, OUTW], mybir.dt.float32, isOutput=True
    )

    # static schedule: subject for each 128-row tile + first/last flags
    sched = []
    for s, ts in enumerate(tiles_per_subject):
        for i in range(ts):
            sched.append((s, i == 0, i == ts - 1))

    with tile.TileContext(nc) as tc:
        with (
            tc.tile_pool(name="xin", bufs=bufs) as xpool,
            tc.tile_pool(name="ps", bufs=1, space=bass.MemorySpace.PSUM) as pspool,
            tc.tile_pool(name="osb", bufs=1) as opool,
        ):
            # subject block stride padded to 128 floats: 4 subjects per 2KB
            # PSUM bank, so a 4-subject drain never reads a bank the PE is
            # still accumulating into
            acc = pspool.tile([64, S, 128], mybir.dt.float32)
            osb = opool.tile([64, S, OUTW], mybir.dt.float32)
            for _rep in range(reps):
                for ch in range(nchunks):
                    xt = xpool.tile([128, ctile * ROWW], mybir.dt.float8e4)
                    nc.sync.dma_start(xt[:], x[ch])
                    for c in range(ctile):
                        s, first, last = sched[ch * ctile + c]
                        mv = xt[:, c * ROWW : c * ROWW + 64]
                        nc.tensor.matmul(
                            acc[0:64, s, 0:64], mv, mv, start=first, stop=last
                        )
                        # drain each PSUM bank group (4 subject blocks) as
                        # soon as its last accumulation lands; on timing
                        # builds accumulate across reps (keeps matmuls live
                        # against DCE), divided out on host
                        if last and s % 4 == 3:
                            g = s - 3
                            if _rep == 0:
                                nc.vector.tensor_copy(
                                    osb[:, g : g + 4, :], acc[:, g : g + 4, 0:OUTW]
                                )
                            else:
                                nc.vector.scalar_tensor_tensor(
                                    osb[:, g : g + 4, :],
                                    acc[:, g : g + 4, 0:OUTW],
                                    1.0,
                                    osb[:, g : g + 4, :],
                                    mybir.AluOpType.mult,
                                    mybir.AluOpType.add,
                                )
            nc.sync.dma_start(out[:], osb[:])
    nc.compile()
    return nc


def _prepare_shards(emb, sid):
    """Group rows by subject, shard across cores, pad to tiles, quantize.

    Returns (shards, counts, sums, tiles_per_subject, nchunks) where sums are
    the per-subject column sums of the QUANTIZED embeddings (fp64 on host),
    consistent with the gram matrices the device computes.
    """
    sid = np.asarray(sid).astype(np.int64).ravel()
    counts = np.bincount(sid, minlength=S).astype(np.int64)
    order = np.argsort(sid, kind="stable")
    starts = np.concatenate([[0], np.cumsum(counts)])

    emb_q = np.asarray(emb, dtype=np.float32).astype(ml_dtypes.float8_e4m3)
    sorted_q = emb_q[order].astype(np.float32)
    sums = np.add.reduceat(sorted_q.astype(np.float64), starts[:-1], axis=0)
    sums[counts == 0] = 0.0

    # per-(core, subject) row counts: split n_s into 8 near-equal parts
    part = np.zeros((NCORES, S), np.int64)
    for s in range(S):
        q, r = divmod(int(counts[s]), NCORES)
        part[:, s] = q
        part[:r, s] += 1
    # tiles per subject: identical across cores (pad shorter parts with zeros)
    tiles_per_subject = [
        max(1, -(-int(part[:, s].max()) // 128)) for s in range(S)
    ]
    T = sum(tiles_per_subject)
    nchunks = NCHUNKS
    # pad the total tile count to a chunk multiple: extra all-zero tiles are
    # appended to subject 15's accumulation group (they contribute zero)
    pad = -T % nchunks
    tiles_per_subject[S - 1] += pad
    T += pad
    ctile = T // nchunks

    tile_base = np.concatenate([[0], np.cumsum(tiles_per_subject)])
    shards = []
    for k in range(NCORES):
        arr = np.zeros((T * 128, ROWW), dtype=ml_dtypes.float8_e4m3)
        for s in range(S):
            off = int(starts[s] + part[:k, s].sum())
            n_ks = int(part[k, s])
            rows = order[off : off + n_ks]
            pos = int(tile_base[s]) * 128
            arr[pos : pos + n_ks, 0:D] = emb_q[rows]
        # chunk-partition-major layout: [nchunks, 128, ctile*ROWW] where
        # dram[ch, p, c*ROWW+e] = row (ch*ctile + c)*128 + p
        arr = np.ascontiguousarray(
            arr.reshape(nchunks, ctile, 128, ROWW).transpose(0, 2, 1, 3)
        ).reshape(nchunks, 128, ctile * ROWW)
        shards.append(arr)
    return shards, counts, sums, tiles_per_subject, nchunks


def _finalize(partials, counts, sums, scale=1.0):
    """Reduce per-core stats and run the tiny [S,S] pairwise stage."""
    tot = np.zeros((64, S, OUTW), np.float64)
    for p in partials:
        q = np.asarray(p, np.float64).reshape(-1, 64, S, OUTW)
        tot += q.sum(axis=0)
    G = (tot * scale).transpose(1, 0, 2)  # [S, 64, 64]
    n = counts.astype(np.float64)

    means = sums / np.maximum(n, 1.0)[:, None]
    denom = np.maximum(n - 1.0, 1.0)[:, None, None]
    cov = (G - n[:, None, None] * means[:, :, None] * means[:, None, :]) / denom
    # (+ lam * I cancels in the pairwise differences, as in the reference)
    iu, ju = np.triu_indices(S, k=1)
    diff = cov[iu] - cov[ju]
    fro2 = np.sum(diff * diff, axis=(1, 2))
    valid = n >= 2.0
    pv = valid[iu] & valid[ju]
    vals = np.sqrt(np.where(pv, fro2, 1.0))
    total = np.sum(np.where(pv, vals, 0.0))
    cnt = int(pv.sum())
    loss = total / max(cnt, 1) if cnt > 0 else 0.0
    return np.float32(loss)


def kernel(embeddings, subject_ids):
    emb = np.asarray(embeddings)
    shards, counts, sums, tiles_per_subject, nchunks = _prepare_shards(
        emb, subject_ids
    )
    nc = _build_nc(tiles_per_subject, nchunks)
    in_maps = [{"x": shards[k]} for k in range(NCORES)]
    res = run_bass_kernel_spmd(nc, in_maps, list(range(NCORES)))
    partials = [res.results[k]["out"] for k in range(NCORES)]
    return _finalize(partials, counts, sums)
